# revision 81
# baseline (speedup 1.0000x reference)
"""Trainium2 Bass kernel for nn_Attention (8-head attention + positional-decay
branch), SPMD across 8 NeuronCores.

Sharding: data-parallel over batch x tensor-parallel over heads.
  core c: batch b = c//4, heads {2*(c%4), 2*(c%4)+1}  (2 "units" per core)

v2 dataflow (per core):
  - bf16 projections (qkvt) and bf16 out2/F2 (positional branch; the decay
    matrix is head-independent so both heads share one 128-partition matmul).
  - fp8e4m3 + DoubleRow matmuls for dots and out1 (0.5 PE cycles/row).
    q/k are evacuated to fp8 and remapped [dh, n] -> [dh%32, dh//32, n] by
    SBUF->SBUF DMA so the 64-deep head contraction becomes 32 partitions x
    2 DoubleRow planes.
  - at = exp(S + C) with a global constant bias C (range-fit for fp8).  Any
    scale on the attention weights cancels in the host-side o1/r division,
    so the softmax stays exact up to dtype noise.
  - exp split across two engines: ACT (table exp -> fp8 at, DoubleRow out1)
    and, for the pairs in M_DVE, DVE (Schraudolph bit-trick exp:
    int16(x*A+B) bitcast to bf16, ~3% max rel err; bf16 out1).
  - softmax denominator r rides out1 as stationary column 64 (constant d).
  - o1 and F2 PSUM tiles are DMA'd straight to HBM (no evacuation pass).
  - host: out1 = o1[0:64]/o1[64], out1-half of to_out, cross-core sum.
"""

import sys

sys.path.insert(0, "/opt/trn_rl_repo")

import numpy as np
import ml_dtypes

import concourse.bass as bass
import concourse.tile as tile
from concourse import bacc, mybir
from concourse.bass_utils import run_bass_kernel_spmd

F32 = mybir.dt.float32
F16 = mybir.dt.float16
BF16 = mybir.dt.bfloat16
F8 = mybir.dt.float8e4
I16 = mybir.dt.int16
EXP = mybir.ActivationFunctionType.Exp
DR = mybir.MatmulPerfMode.DoubleRow

N = 2048          # sequence length
DH = 64           # head dim
B = 2             # batch
KT = 4            # dim // 128 contraction tiles
NI = 16           # n // 128 j-tiles
NP = 8            # n // 256 j-tile pairs
NCORES = 8

# (ich, u, pair) units whose exp runs on DVE (Schraudolph/bf16 route).
# Checkerboarded over (u, pair) so ACT always has the sibling unit's exp.
M_DVE = frozenset(
    {(i, 0, p) for i in range(2) for p in (1, 3, 5)}
    | {(i, 1, p) for i in range(2) for p in (2, 4)}
    | {(0, 1, 6), (1, 1, 6)}
)

# global exp bias: at = exp(S + C_BIAS) <= exp(SMAX + C_BIAS) ~ 170 < 240
C_BIAS = float(np.log(170.0) - 6.2)
# Schraudolph bf16-bitcast exp: i16 = int(x*A + B); +0.5 centers the
# truncating float->int conversion.
SCH_A = 128.0 / np.log(2.0)
SCH_B = 127.0 * 128.0 - 0.0301975 * 128.0 + 0.5 + SCH_A * C_BIAS
VSCALE = 0.25     # v'' = v * VSCALE; d column = VSCALE (cancels in o1/r)


def build_program() -> bass.Bass:
    nc = bacc.Bacc(None)

    xt_d = nc.declare_dram_parameter("xt", [KT, 128, N], BF16, False)
    # fp8 copies for the q/k projection (DoubleRow over the contraction):
    # xt8[t, p, pl, n] = x.T[t*256 + pl*128 + p, n];  wqk8 cols =
    # [k-lo64 | k-hi64 | q-lo64 | q-hi64], lo/hi = head-dim halves,
    # each 64 = [u0(32) | u1(32)], pre-scaled by 8/sqrt(8).
    xt8_d = nc.declare_dram_parameter("xt8", [128, 4, N], F8, False)
    wqk8_d = nc.declare_dram_parameter("wqk8", [128, 4, 256], F8, False)
    # cst packs [wvt (4x256) | gb (4x256) | w2s (512)] column-blocks
    cst_d = nc.declare_dram_parameter("cst", [128, 2048], BF16, False)
    rstc_d = nc.declare_dram_parameter("rstc", [128, 2], F32, False)
    o1t0_d = nc.declare_dram_parameter("o1t0", [65, N], F16, isOutput=True)
    o1t1_d = nc.declare_dram_parameter("o1t1", [65, N], F16, isOutput=True)
    o2_d = nc.declare_dram_parameter("o2", [128, N], BF16, isOutput=True)
    o1t_d = [o1t0_d, o1t1_d]

    with tile.TileContext(nc) as tc:
        with (
            tc.tile_pool(name="const", bufs=1) as cp,
            tc.tile_pool(name="at8p", bufs=20) as ap8,
            tc.tile_pool(name="atip", bufs=9) as api,
            tc.tile_pool(name="psum", bufs=1, space="PSUM") as pp,
        ):
            # ---- resident SBUF tensors ----
            xt_sb = cp.tile([128, KT, N], BF16, name="xt_sb")
            xt8_sb = cp.tile([128, 4, N], F8, name="xt8_sb")
            wqk8_sb = cp.tile([128, 4, 256], F8, name="wqk8_sb")
            cst_sb = cp.tile([128, 2048], BF16, name="cst_sb")

            def wvt_c(kt):
                return cst_sb[:, kt * 256:(kt + 1) * 256]

            def g_c(gi):
                return cst_sb[:, 1024 + gi * 256:1024 + (gi + 1) * 256]

            rstc_sb = cp.tile([128, 2], F32, name="rstc_sb")
            q8 = cp.tile([64, 2, N], F8, name="q8")
            k8 = cp.tile([64, 2, N], F8, name="k8")
            # v''/d stationary, fp8 DR layout: [p=j, u, jtile, 128] with
            # cols 0:64 = v*VSCALE, col 64 = VSCALE
            vd8 = cp.tile([128, 2, NI, 128], F8, name="vd8")
            # bf16 copy for the DVE-exp route
            vd16 = cp.tile([128, 2, NI, 66], BF16, name="vd16")
            # t' = t/s_j for out2 (cols 0:64 head u0, 64:128 head u1)
            vtt = cp.tile([128, NI, 128], BF16, name="vtt")
            o2sb = cp.tile([128, N], BF16, name="o2sb")
            o1sb = [cp.tile([65, N], F16, name=f"o1sb{u}") for u in range(2)]

            # warm the ACT exp table off the critical path
            warm = cp.tile([1, 8], F32, name="warm")
            nc.vector.memset(warm[:], 0.0)
            nc.scalar.activation(warm[:], warm[:], EXP)
            # denominator columns (vd16's col 64 comes from the gpsimd copy)
            nc.vector.memset(vd8[:, :, :, 64:65], VSCALE)
            # per-partition exp bias column (scalar consts need an AP)
            cbias = cp.tile([128, 1], F32, name="cbias")
            nc.vector.memset(cbias[:], C_BIAS)
            # PE p-state warm-up: a stream of tiny matmuls keeps the tensor
            # engine busy through the input-DMA window so the real prologue
            # runs at full clock (the cost model's ramp needs ~3us busy)
            dmy = cp.tile([128, 16], BF16, name="dmy")
            nc.vector.memset(dmy[:], 0.0)
            psw = pp.tile([16, 16], F32, tag="E", bufs=1, name="psw")
            for _ in range(220):
                nc.tensor.matmul(psw, lhsT=dmy[:, 0:16], rhs=dmy[:],
                                 start=True, stop=True,
                                 skip_group_check=True)

            # ---- input DMAs (critical-path first) ----
            def dma_xt(c4):
                nc.sync.dma_start(
                    out=xt_sb[:, :, c4 * 512:(c4 + 1) * 512],
                    in_=xt_d[:, :, c4 * 512:(c4 + 1) * 512]
                    .rearrange("k p c -> p k c"))

            def dma_xt8(c4):
                nc.sync.dma_start(
                    out=xt8_sb[:, :, c4 * 512:(c4 + 1) * 512],
                    in_=xt8_d[:, :, c4 * 512:(c4 + 1) * 512])

            dma_xt8(0)
            nc.sync.dma_start(out=wqk8_sb[:], in_=wqk8_d[:])
            for c4 in range(1, 4):
                dma_xt8(c4)

            # evac scale recovering S = q.k/8 from the 8x-ranged fp8 weights
            QKS = float(8.0 ** -1.5)
            qks = cp.tile([128, 1], F32, name="qks")
            nc.vector.memset(qks[:], QKS)

            # ---- emit helpers.  PSUM tags: A0,A1 1 bank; E,Fq,Bq 2 ----
            qk_slot = [0]
            QK_SLOTS = ("A0", "A1", "Bq", "E", "Fq")

            def emit_qk_chunk(wcol, dst, c4):
                """fp8-DR projection chunk straight into the dots layout:
                psum partitions = [u0(32) | u1(32)] of one head-dim half.
                Psums rotate over 5 bank slots (the fill banks are still
                free this early) so chunks never wait on evacuations."""
                for half in range(2):
                    qk_slot[0] = (qk_slot[0] + 1) % 5
                    tag = QK_SLOTS[qk_slot[0]]
                    ps = pp.tile([64, 512], F32, tag=tag, bufs=1,
                                 name="qk_ps")
                    for ic in range(2):
                        for ktp in range(2):
                            nc.tensor.matmul(
                                ps[:, ic * 256:(ic + 1) * 256],
                                lhsT=wqk8_sb[:, 2 * ktp:2 * ktp + 2,
                                             wcol + half * 64:
                                             wcol + half * 64 + 64],
                                rhs=xt8_sb[:, 2 * ktp:2 * ktp + 2,
                                           c4 * 512 + ic * 256:
                                           c4 * 512 + (ic + 1) * 256],
                                start=(ktp == 0), stop=(ktp == 1),
                                perf_mode=DR, skip_group_check=True)
                    out = dst[0:64, half, c4 * 512:(c4 + 1) * 512]
                    if half == 0 or c4 >= 2:
                        nc.vector.tensor_scalar_mul(out, ps[:], QKS)
                    else:
                        nc.scalar.activation(
                            out, ps[:], mybir.ActivationFunctionType.Copy,
                            scale=qks[0:64, :])

            def emit_vt(ip, tag):
                # two j-tiles (2ip, 2ip+1) per 1-bank psum
                ps = pp.tile([128, 2, 256], F32, tag=tag, bufs=1,
                             name="vt_ps")
                for half in range(2):
                    ib = 2 * ip + half
                    for kt in range(KT):
                        nc.tensor.matmul(
                            ps[:, half, :],
                            lhsT=xt_sb[:, kt, ib * 128:(ib + 1) * 128],
                            rhs=wvt_c(kt),
                            start=(kt == 0 and half == 0), stop=(kt == KT - 1),
                            skip_group_check=True)
                # t' -> vtt bf16 (1/s_max folded into w2s; only the edge
                # j-tiles 0 and 15 need the per-j correction column)
                if ip == 0:
                    nc.vector.tensor_scalar_mul(
                        vtt[:, 0, :], ps[:, 0, 0:128], rstc_sb[:, 0:1])
                    nc.vector.tensor_copy(vtt[:, 1, :], ps[:, 1, 0:128])
                elif ip == NP - 1:
                    nc.vector.tensor_copy(vtt[:, 14, :], ps[:, 0, 0:128])
                    nc.vector.tensor_scalar_mul(
                        vtt[:, 15, :], ps[:, 1, 0:128], rstc_sb[:, 1:2])
                else:
                    nc.vector.tensor_copy(
                        vtt[:, 2 * ip:2 * ip + 2, :], ps[:, :, 0:128])
                vsrc = ps[:, :, 128:256].rearrange("p i (u f) -> p i u f",
                                                   u=2)
                nc.vector.tensor_copy(
                    vd8[:, :, 2 * ip:2 * ip + 2, 0:64]
                    .rearrange("p u i f -> p i u f"), vsrc)

            at_tiles = {}
            ef_rot = [0]
            EFB = ("E", "Fq", "Bq")

            def emit_fill(ich, p, u, h):
                """dots psum fill [128 j(2 planes), 512 i] + exp."""
                ef_rot[0] = (ef_rot[0] + 1) % 3
                ps = pp.tile([128, 2, 512], F32, tag=EFB[ef_rot[0]], bufs=1,
                             name="st_ps")
                i0 = ich * 1024 + h * 512
                for pl in range(2):
                    for ic in range(2):
                        nc.tensor.matmul(
                            ps[:, pl, ic * 256:(ic + 1) * 256],
                            lhsT=k8[u * 32:(u + 1) * 32, :,
                                    (p * 2 + pl) * 128:(p * 2 + pl + 1) * 128],
                            rhs=q8[u * 32:(u + 1) * 32, :,
                                   i0 + ic * 256:i0 + (ic + 1) * 256],
                            start=(ic == 0), stop=(ic == 1),
                            perf_mode=DR, skip_group_check=True)
                key = (ich, u, p)
                if key in M_DVE:
                    if key not in at_tiles:
                        at_tiles[key] = api.tile([128, 2, 1024], I16,
                                                 tag="ati", name="ati")
                    at = at_tiles[key]
                    nc.vector.tensor_scalar(
                        out=at[:, :, h * 512:(h + 1) * 512], in0=ps[:],
                        scalar1=SCH_A, scalar2=SCH_B,
                        op0=mybir.AluOpType.mult, op1=mybir.AluOpType.add)
                else:
                    if key not in at_tiles:
                        at_tiles[key] = ap8.tile([128, 2, 1024], F8,
                                                 tag="at8", name="at8")
                    at = at_tiles[key]
                    nc.scalar.activation(
                        at[:, :, h * 512:(h + 1) * 512], ps[:], EXP,
                        bias=cbias[:])

            o1ps = {}

            def emit_out1(ich, u, p):
                key = (ich, u, p)
                at = at_tiles.pop(key)
                # the last accumulator pair rides the freed fill banks so
                # it can run concurrently with out1(ich1, u0) on A
                tags = ("E", "Fq") if (ich, u) == (1, 1) else ("A0", "A1")
                for hb in range(2):
                    if (u, ich, hb) not in o1ps:
                        o1ps[(u, ich, hb)] = pp.tile(
                            [65, 512], F32, tag=tags[hb], bufs=1,
                            name=f"o1_ps{u}{hb}")
                if key in M_DVE:
                    for pl in range(2):
                        for hb in range(2):
                            nc.tensor.matmul(
                                o1ps[(u, ich, hb)],
                                lhsT=vd16[:, u, p * 2 + pl, 0:65],
                                rhs=at[:, pl,
                                       hb * 512:(hb + 1) * 512].bitcast(BF16),
                                start=False, stop=(p == NP - 1 and pl == 1),
                                skip_group_check=True)
                else:
                    for hb in range(2):
                        for ic in range(2):
                            nc.tensor.matmul(
                                o1ps[(u, ich, hb)][:, ic * 256:(ic + 1) * 256],
                                lhsT=vd8[:, u, p * 2:p * 2 + 2, 0:65],
                                rhs=at[:, :,
                                       hb * 512 + ic * 256:
                                       hb * 512 + (ic + 1) * 256],
                                start=(p == 0 and ic == 0),
                                stop=(p == NP - 1),
                                perf_mode=DR, skip_group_check=True)

            def emit_o1_dma(ich, u):
                for hb in range(2):
                    ps = o1ps.pop((u, ich, hb))
                    dst = o1sb[u][:, ich * 1024 + hb * 512:
                                  ich * 1024 + (hb + 1) * 512]
                    if hb == 0:
                        nc.vector.tensor_copy(dst, ps[:])
                    else:
                        nc.scalar.copy(dst, ps[:])
                nc.sync.dma_start(
                    out=o1t_d[u][:, ich * 1024:(ich + 1) * 1024],
                    in_=o1sb[u][:, ich * 1024:(ich + 1) * 1024])

            def emit_out2(c, tag):
                ps = pp.tile([128, 256], F32, tag=tag, bufs=1, name="o2_ps")
                jts = [jt for jt in range(2 * c - 1, 2 * c + 3)
                       if 0 <= jt < NI]
                for idx, jt in enumerate(jts):
                    nc.tensor.matmul(
                        ps, lhsT=vtt[:, jt, :],
                        rhs=g_c(jt - 2 * c + 1),
                        start=(idx == 0), stop=(idx == len(jts) - 1))
                nc.vector.tensor_copy(o2sb[:, c * 256:(c + 1) * 256], ps[:])
                if c in (3, NP - 1):
                    cs = slice((c - 3) * 256, (c + 1) * 256)
                    nc.sync.dma_start(out=o2_d[:, cs], in_=o2sb[:, cs])

            # ---- prologue: fp8-DR qk chunks land directly in dots layout;
            # early fills start as soon as k/q chunk 0 are evacuated ----
            emit_qk_chunk(0, k8, 0)
            emit_qk_chunk(128, q8, 0)
            for c4 in range(4):
                dma_xt(c4)
            nc.sync.dma_start(out=cst_sb[:], in_=cst_d[:])
            nc.sync.dma_start(out=rstc_sb[:], in_=rstc_d[:])
            for p in range(2):
                for u in range(2):
                    emit_fill(0, p, u, 0)
            emit_qk_chunk(0, k8, 1)
            emit_qk_chunk(128, q8, 1)
            for p in range(2):
                for u in range(2):
                    emit_fill(0, p, u, 1)
            emit_qk_chunk(0, k8, 2)
            emit_qk_chunk(128, q8, 2)
            emit_qk_chunk(0, k8, 3)
            emit_qk_chunk(128, q8, 3)

            # ---- main: remaining ich0 fills + vt + out2 + F2 ----
            # Tag timelines (serialized by WAR deps within each tag):
            #   A0/A1: qk -> vt -> out2 -> F2 -> out1 (all u/ich, serial)
            #   E/Fq/Bq: warmup + all dots fills (triple-buffered)
            for p in range(NP):
                fp = p + 2
                if fp < NP:
                    for h in range(2):
                        for u in range(2):
                            emit_fill(0, fp, u, h)
                else:
                    # start ich1 fills early to keep the exp engines fed
                    for h in range(2):
                        for u in range(2):
                            emit_fill(1, fp - NP, u, h)
                emit_vt(p, "A0" if p % 2 == 0 else "A1")
                if p >= 1:
                    emit_out2(p - 1, "A0" if p % 2 == 1 else "A1")
            # bf16 stationary copy for the DVE-route out1 (Pool engine;
            # sources the fp8 values so both routes see the same v)
            nc.gpsimd.tensor_copy(vd16[:, :, :, 0:65], vd8[:, :, :, 0:65])
            emit_out2(NP - 1, "A1")
            # ---- remaining ich1 fills ----
            for s in range(NP - 2):
                fp = s + 2
                for h in range(2):
                    for u in range(2):
                        emit_fill(1, fp, u, h)
            # ---- out1, serial on the A banks: (ich0,u0) .. (ich1,u1) ----
            for ich in range(2):
                for u in range(2):
                    for p in range(NP):
                        emit_out1(ich, u, p)
                    emit_o1_dma(ich, u)

    nc.finalize()
    return nc


_PROGRAM = None


def _get_program():
    global _PROGRAM
    if _PROGRAM is None:
        _PROGRAM = build_program()
    return _PROGRAM


def _host_tables():
    d = np.arange(N, dtype=np.float64)
    g = np.exp(-d / np.e)
    cum = np.cumsum(g)
    j = np.arange(N)
    s = cum[j] + cum[N - 1 - j] - g[0]        # s[j] = sum_k exp(-|j-k|/e)
    smax = s.max()
    # edge corrections s_max/s_j for j-tiles 0 and 15 (elsewhere s_j = s_max
    # to ~1e-10 relative, folded into w2s as the global 1/s_max)
    rstc = np.ascontiguousarray(np.stack(
        [smax / s[0:128], smax / s[N - 128:N]], axis=1).astype(np.float32))
    gi = np.arange(4)[:, None, None]
    p = np.arange(128)[None, :, None]
    f = np.arange(256)[None, None, :]
    gb = np.ascontiguousarray(
        np.exp(-np.abs(f - p - (gi - 1) * 128) / np.e)
        .astype(ml_dtypes.bfloat16))
    return rstc, gb, smax


_TABLES = None


def _tables():
    global _TABLES
    if _TABLES is None:
        _TABLES = _host_tables()
    return _TABLES


def make_in_maps(x, w_qkv, w_out, b_out):
    x = np.asarray(x, np.float32)
    w_qkv = np.asarray(w_qkv, np.float32)
    w_out = np.asarray(w_out, np.float32)
    rstc, gb, smax = _tables()
    w8_scale = 8.0                       # fp8 range scale, undone at evac

    wq_full = w_qkv[0:512]
    wk_full = w_qkv[512:1024]
    wv_full = w_qkv[1024:1536]
    wt_full = w_qkv[1536:2048]

    bf = ml_dtypes.bfloat16
    in_maps = []
    for c in range(NCORES):
        b = c // 4
        h0 = 2 * (c % 4)
        h1 = h0 + 1

        f8 = ml_dtypes.float8_e4m3
        xT = x[b].T                                        # [512, N]
        xt = np.ascontiguousarray(xT.reshape(KT, 128, N).astype(bf))
        xt8 = np.ascontiguousarray(
            xT.reshape(2, 2, 128, N).transpose(2, 0, 1, 3)
            .reshape(128, 4, N).astype(f8))

        # wqk8 cols: [k-lo | k-hi | q-lo | q-hi], each 64 = [u0 | u1]
        def whalf(wfull, h, lohi):
            return wfull[h * 64 + lohi * 32:h * 64 + lohi * 32 + 32].T

        cols = []
        for wfull in (wk_full, wq_full):
            for lohi in range(2):
                cols += [whalf(wfull, h0, lohi), whalf(wfull, h1, lohi)]
        wqk8_ = np.concatenate(cols, axis=1) * w8_scale    # [512, 256]
        wqk8 = np.ascontiguousarray(
            wqk8_.reshape(2, 2, 128, 256).transpose(2, 0, 1, 3)
            .reshape(128, 4, 256).astype(f8))
        wvt_ = np.concatenate(
            [wt_full[h0 * 64:(h0 + 1) * 64].T,
             wt_full[h1 * 64:(h1 + 1) * 64].T,
             wv_full[h0 * 64:(h0 + 1) * 64].T * VSCALE,
             wv_full[h1 * 64:(h1 + 1) * 64].T * VSCALE], axis=1)
        wvt = wvt_.reshape(KT, 128, 256)            # [kt, p, c]
        wvt_cols = wvt.transpose(1, 0, 2).reshape(128, 1024)
        gb_cols = np.asarray(gb, np.float32).transpose(1, 0, 2).reshape(128, 1024)
        cst = np.ascontiguousarray(np.concatenate(
            [wvt_cols, gb_cols], axis=1).astype(bf))
        in_maps.append({
            "xt": xt, "xt8": xt8, "wqk8": wqk8, "cst": cst, "rstc": rstc,
        })
    return in_maps


def combine_outputs(results, w_out, b_out):
    """Host-side unshard: per-core partials -> full [B, N, DIM] output."""
    w_out = np.asarray(w_out, np.float32)
    b_out = np.asarray(b_out, np.float32)
    out = np.zeros((B, N, 512), np.float64)
    for c in range(NCORES):
        r = results[c]
        b = c // 4
        h0 = 2 * (c % 4)
        rstc, gb, smax = _tables()
        o2 = r["o2"].astype(np.float64)                    # [128 ch, N]
        w2 = np.concatenate(
            [w_out[:, h0 * 128 + 64:(h0 + 1) * 128].T,
             w_out[:, (h0 + 1) * 128 + 64:(h0 + 2) * 128].T],
            axis=0).astype(np.float64) / smax
        part = o2.T @ w2                                   # [N, 512]
        for u, h in ((0, h0), (1, h0 + 1)):
            o1 = r[f"o1t{u}"].astype(np.float64)
            o1v = (o1[0:64] / o1[64][None, :]).T            # [N, 64]
            w1 = w_out[:, h * 128:h * 128 + 64].T.astype(np.float64)
            part = part + o1v @ w1
        out[b] += part
    out += b_out[None, None, :].astype(np.float64)
    return out.astype(np.float32)


def kernel(x, w_qkv, w_out, b_out):
    nc = _get_program()
    in_maps = make_in_maps(x, w_qkv, w_out, b_out)
    res = run_bass_kernel_spmd(nc, in_maps, core_ids=list(range(NCORES)))
    return combine_outputs(res.results, w_out, b_out)


def kernel_profiled(x, w_qkv, w_out, b_out):
    out = kernel(x, w_qkv, w_out, b_out)
    return out, None


# revision 85
# speedup vs baseline: 1.0375x; 1.0375x over previous
"""Trainium2 Bass kernel for nn_Attention (8-head attention + positional-decay
branch), SPMD across 8 NeuronCores.

Sharding: data-parallel over batch x tensor-parallel over heads.
  core c: batch b = c//4, heads {2*(c%4), 2*(c%4)+1}  (2 "units" per core)

v2 dataflow (per core):
  - bf16 projections (qkvt) and bf16 out2/F2 (positional branch; the decay
    matrix is head-independent so both heads share one 128-partition matmul).
  - fp8e4m3 + DoubleRow matmuls for dots and out1 (0.5 PE cycles/row).
    q/k are evacuated to fp8 and remapped [dh, n] -> [dh%32, dh//32, n] by
    SBUF->SBUF DMA so the 64-deep head contraction becomes 32 partitions x
    2 DoubleRow planes.
  - at = exp(S + C) with a global constant bias C (range-fit for fp8).  Any
    scale on the attention weights cancels in the host-side o1/r division,
    so the softmax stays exact up to dtype noise.
  - exp split across two engines: ACT (table exp -> fp8 at, DoubleRow out1)
    and, for the pairs in M_DVE, DVE (Schraudolph bit-trick exp:
    int16(x*A+B) bitcast to bf16, ~3% max rel err; bf16 out1).
  - softmax denominator r rides out1 as stationary column 64 (constant d).
  - o1 and F2 PSUM tiles are DMA'd straight to HBM (no evacuation pass).
  - host: out1 = o1[0:64]/o1[64], out1-half of to_out, cross-core sum.
"""

import sys

sys.path.insert(0, "/opt/trn_rl_repo")

import numpy as np
import ml_dtypes

import concourse.bass as bass
import concourse.tile as tile
from concourse import bacc, mybir
from concourse.bass_utils import run_bass_kernel_spmd

F32 = mybir.dt.float32
F16 = mybir.dt.float16
BF16 = mybir.dt.bfloat16
F8 = mybir.dt.float8e4
I16 = mybir.dt.int16
EXP = mybir.ActivationFunctionType.Exp
DR = mybir.MatmulPerfMode.DoubleRow

N = 2048          # sequence length
DH = 64           # head dim
B = 2             # batch
KT = 4            # dim // 128 contraction tiles
NI = 16           # n // 128 j-tiles
NP = 8            # n // 256 j-tile pairs
NCORES = 8

# (ich, u, pair) units whose exp runs on DVE (Schraudolph/bf16 route).
# Checkerboarded over (u, pair) so ACT always has the sibling unit's exp.
M_DVE = frozenset(
    {(i, 0, p) for i in range(2) for p in (1, 3, 5)}
    | {(i, 1, p) for i in range(2) for p in (2, 4)}
    | {(0, 1, 6), (1, 1, 6)}
)

# global exp bias: at = exp(S + C_BIAS) <= exp(SMAX + C_BIAS) ~ 170 < 240
C_BIAS = float(np.log(170.0) - 6.2)
# Schraudolph bf16-bitcast exp: i16 = int(x*A + B); +0.5 centers the
# truncating float->int conversion.
SCH_A = 128.0 / np.log(2.0)
SCH_B = 127.0 * 128.0 - 0.0301975 * 128.0 + 0.5 + SCH_A * C_BIAS
VSCALE = 0.25     # v'' = v * VSCALE; d column = VSCALE (cancels in o1/r)


def build_program() -> bass.Bass:
    nc = bacc.Bacc(None)

    xt_d = nc.declare_dram_parameter("xt", [KT, 128, N], BF16, False)
    # fp8 copies for the q/k projection (DoubleRow over the contraction):
    # xt8[t, p, pl, n] = x.T[t*256 + pl*128 + p, n];  wqk8 cols =
    # [k-lo64 | k-hi64 | q-lo64 | q-hi64], lo/hi = head-dim halves,
    # each 64 = [u0(32) | u1(32)], pre-scaled by 8/sqrt(8).
    xt8_d = nc.declare_dram_parameter("xt8", [128, 4, N], F8, False)
    wqk8_d = nc.declare_dram_parameter("wqk8", [128, 4, 256], F8, False)
    # cst packs [wvt (4x256) | gb (4x256) | w2s (512)] column-blocks
    cst_d = nc.declare_dram_parameter("cst", [128, 2048], BF16, False)
    rstc_d = nc.declare_dram_parameter("rstc", [128, 2], F32, False)
    o1t0_d = nc.declare_dram_parameter("o1t0", [65, N], F16, isOutput=True)
    o1t1_d = nc.declare_dram_parameter("o1t1", [65, N], F16, isOutput=True)
    o2_d = nc.declare_dram_parameter("o2", [128, N], BF16, isOutput=True)
    o1t_d = [o1t0_d, o1t1_d]

    with tile.TileContext(nc) as tc:
        with (
            tc.tile_pool(name="const", bufs=1) as cp,
            tc.tile_pool(name="at8p", bufs=20) as ap8,
            tc.tile_pool(name="atip", bufs=9) as api,
            tc.tile_pool(name="psum", bufs=1, space="PSUM") as pp,
        ):
            # ---- resident SBUF tensors ----
            xt_sb = cp.tile([128, KT, N], BF16, name="xt_sb")
            xt8_sb = cp.tile([128, 4, N], F8, name="xt8_sb")
            wqk8_sb = cp.tile([128, 4, 256], F8, name="wqk8_sb")
            cst_sb = cp.tile([128, 2048], BF16, name="cst_sb")

            def wvt_c(kt):
                return cst_sb[:, kt * 256:(kt + 1) * 256]

            def g_c(gi):
                return cst_sb[:, 1024 + gi * 256:1024 + (gi + 1) * 256]

            rstc_sb = cp.tile([128, 2], F32, name="rstc_sb")
            q8 = cp.tile([64, 2, N], F8, name="q8")
            k8 = cp.tile([64, 2, N], F8, name="k8")
            # v''/d stationary, fp8 DR layout: [p=j, u, jtile, 128] with
            # cols 0:64 = v*VSCALE, col 64 = VSCALE
            vd8 = cp.tile([128, 2, NI, 128], F8, name="vd8")
            # bf16 copy for the DVE-exp route
            vd16 = cp.tile([128, 2, NI, 66], BF16, name="vd16")
            # t' = t/s_j for out2 (cols 0:64 head u0, 64:128 head u1)
            vtt = cp.tile([128, NI, 128], BF16, name="vtt")
            o2sb = cp.tile([128, N], BF16, name="o2sb")
            o1sb = [cp.tile([65, N], F16, name=f"o1sb{u}") for u in range(2)]

            # warm the ACT exp table off the critical path
            warm = cp.tile([1, 8], F32, name="warm")
            nc.vector.memset(warm[:], 0.0)
            nc.scalar.activation(warm[:], warm[:], EXP)
            # denominator columns (vd16's col 64 comes from the gpsimd copy)
            nc.vector.memset(vd8[:, :, :, 64:65], VSCALE)
            # per-partition exp bias column (scalar consts need an AP)
            cbias = cp.tile([128, 1], F32, name="cbias")
            nc.vector.memset(cbias[:], C_BIAS)
            # PE p-state warm-up: a stream of tiny matmuls keeps the tensor
            # engine busy through the input-DMA window so the real prologue
            # runs at full clock (the cost model's ramp needs ~3us busy)
            dmy = cp.tile([128, 16], BF16, name="dmy")
            nc.vector.memset(dmy[:], 0.0)
            psw = pp.tile([16, 16], F32, tag="E", bufs=1, name="psw")
            for _ in range(220):
                nc.tensor.matmul(psw, lhsT=dmy[:, 0:16], rhs=dmy[:],
                                 start=True, stop=True,
                                 skip_group_check=True)

            # ---- input DMAs (critical-path first) ----
            def dma_xt(c4):
                nc.sync.dma_start(
                    out=xt_sb[:, :, c4 * 512:(c4 + 1) * 512],
                    in_=xt_d[:, :, c4 * 512:(c4 + 1) * 512]
                    .rearrange("k p c -> p k c"))

            def dma_xt8(c4):
                nc.sync.dma_start(
                    out=xt8_sb[:, :, c4 * 512:(c4 + 1) * 512],
                    in_=xt8_d[:, :, c4 * 512:(c4 + 1) * 512])

            dma_xt8(0)
            nc.sync.dma_start(out=wqk8_sb[:], in_=wqk8_d[:])
            for c4 in range(1, 4):
                dma_xt8(c4)

            # evac scale recovering S = q.k/8 from the 8x-ranged fp8 weights
            QKS = float(8.0 ** -1.5)
            qks = cp.tile([128, 1], F32, name="qks")
            nc.vector.memset(qks[:], QKS)

            # ---- emit helpers.  PSUM tags: A0,A1 1 bank; E,Fq,Bq 2 ----
            qk_slot = [0]
            QK_SLOTS = ("A0", "A1", "Bq", "E", "Fq")

            def emit_qk_chunk(wcol, dst, c4):
                """fp8-DR projection chunk straight into the dots layout:
                psum partitions = [u0(32) | u1(32)] of one head-dim half.
                Psums rotate over 5 bank slots (the fill banks are still
                free this early) so chunks never wait on evacuations."""
                for half in range(2):
                    qk_slot[0] = (qk_slot[0] + 1) % 5
                    tag = QK_SLOTS[qk_slot[0]]
                    ps = pp.tile([64, 512], F32, tag=tag, bufs=1,
                                 name="qk_ps")
                    for ic in range(2):
                        for ktp in range(2):
                            nc.tensor.matmul(
                                ps[:, ic * 256:(ic + 1) * 256],
                                lhsT=wqk8_sb[:, 2 * ktp:2 * ktp + 2,
                                             wcol + half * 64:
                                             wcol + half * 64 + 64],
                                rhs=xt8_sb[:, 2 * ktp:2 * ktp + 2,
                                           c4 * 512 + ic * 256:
                                           c4 * 512 + (ic + 1) * 256],
                                start=(ktp == 0), stop=(ktp == 1),
                                perf_mode=DR, skip_group_check=True)
                    out = dst[0:64, half, c4 * 512:(c4 + 1) * 512]
                    if half == 0 or c4 >= 2:
                        nc.vector.tensor_scalar_mul(out, ps[:], QKS)
                    else:
                        nc.scalar.activation(
                            out, ps[:], mybir.ActivationFunctionType.Copy,
                            scale=qks[0:64, :])

            def emit_vt(ip, tag):
                # two j-tiles (2ip, 2ip+1) per 1-bank psum
                ps = pp.tile([128, 2, 256], F32, tag=tag, bufs=1,
                             name="vt_ps")
                for half in range(2):
                    ib = 2 * ip + half
                    for kt in range(KT):
                        nc.tensor.matmul(
                            ps[:, half, :],
                            lhsT=xt_sb[:, kt, ib * 128:(ib + 1) * 128],
                            rhs=wvt_c(kt),
                            start=(kt == 0 and half == 0), stop=(kt == KT - 1),
                            skip_group_check=True)
                # t' -> vtt bf16 (1/s_max folded into w2s; only the edge
                # j-tiles 0 and 15 need the per-j correction column)
                if ip == 0:
                    nc.vector.tensor_scalar_mul(
                        vtt[:, 0, :], ps[:, 0, 0:128], rstc_sb[:, 0:1])
                    nc.vector.tensor_copy(vtt[:, 1, :], ps[:, 1, 0:128])
                elif ip == NP - 1:
                    nc.vector.tensor_copy(vtt[:, 14, :], ps[:, 0, 0:128])
                    nc.vector.tensor_scalar_mul(
                        vtt[:, 15, :], ps[:, 1, 0:128], rstc_sb[:, 1:2])
                else:
                    nc.vector.tensor_copy(
                        vtt[:, 2 * ip:2 * ip + 2, :], ps[:, :, 0:128])
                vsrc = ps[:, :, 128:256].rearrange("p i (u f) -> p i u f",
                                                   u=2)
                nc.vector.tensor_copy(
                    vd8[:, :, 2 * ip:2 * ip + 2, 0:64]
                    .rearrange("p u i f -> p i u f"), vsrc)

            at_tiles = {}
            ef_rot = [0]
            EFB = ("E", "Fq", "Bq")

            def emit_fill(ich, p, u, h):
                """dots psum fill [128 j(2 planes), 512 i] + exp."""
                ef_rot[0] = (ef_rot[0] + 1) % 3
                ps = pp.tile([128, 2, 512], F32, tag=EFB[ef_rot[0]], bufs=1,
                             name="st_ps")
                i0 = ich * 1024 + h * 512
                for pl in range(2):
                    for ic in range(2):
                        nc.tensor.matmul(
                            ps[:, pl, ic * 256:(ic + 1) * 256],
                            lhsT=k8[u * 32:(u + 1) * 32, :,
                                    (p * 2 + pl) * 128:(p * 2 + pl + 1) * 128],
                            rhs=q8[u * 32:(u + 1) * 32, :,
                                   i0 + ic * 256:i0 + (ic + 1) * 256],
                            start=(ic == 0), stop=(ic == 1),
                            perf_mode=DR, skip_group_check=True)
                key = (ich, u, p)
                if key in M_DVE:
                    if key not in at_tiles:
                        at_tiles[key] = api.tile([128, 2, 1024], I16,
                                                 tag="ati", name="ati")
                    at = at_tiles[key]
                    nc.vector.tensor_scalar(
                        out=at[:, :, h * 512:(h + 1) * 512], in0=ps[:],
                        scalar1=SCH_A, scalar2=SCH_B,
                        op0=mybir.AluOpType.mult, op1=mybir.AluOpType.add)
                else:
                    if key not in at_tiles:
                        at_tiles[key] = ap8.tile([128, 2, 1024], F8,
                                                 tag="at8", name="at8")
                    at = at_tiles[key]
                    nc.scalar.activation(
                        at[:, :, h * 512:(h + 1) * 512], ps[:], EXP,
                        bias=cbias[:])

            o1ps = {}

            def emit_out1(ich, u, p):
                key = (ich, u, p)
                at = at_tiles.pop(key)
                # the last accumulator pair rides the freed fill banks so
                # it can run concurrently with out1(ich1, u0) on A
                tags = ("E", "Fq") if (ich, u) == (1, 1) else ("A0", "A1")
                for hb in range(2):
                    if (u, ich, hb) not in o1ps:
                        o1ps[(u, ich, hb)] = pp.tile(
                            [65, 512], F32, tag=tags[hb], bufs=1,
                            name=f"o1_ps{u}{hb}")
                if key in M_DVE:
                    for pl in range(2):
                        for hb in range(2):
                            nc.tensor.matmul(
                                o1ps[(u, ich, hb)],
                                lhsT=vd16[:, u, p * 2 + pl, 0:65],
                                rhs=at[:, pl,
                                       hb * 512:(hb + 1) * 512].bitcast(BF16),
                                start=False, stop=(p == NP - 1 and pl == 1),
                                skip_group_check=True)
                else:
                    for hb in range(2):
                        for ic in range(2):
                            nc.tensor.matmul(
                                o1ps[(u, ich, hb)][:, ic * 256:(ic + 1) * 256],
                                lhsT=vd8[:, u, p * 2:p * 2 + 2, 0:65],
                                rhs=at[:, :,
                                       hb * 512 + ic * 256:
                                       hb * 512 + (ic + 1) * 256],
                                start=(p == 0 and ic == 0),
                                stop=(p == NP - 1),
                                perf_mode=DR, skip_group_check=True)

            def emit_o1_dma(ich, u):
                for hb in range(2):
                    ps = o1ps.pop((u, ich, hb))
                    dst = o1sb[u][:, ich * 1024 + hb * 512:
                                  ich * 1024 + (hb + 1) * 512]
                    if hb == 0:
                        nc.vector.tensor_copy(dst, ps[:])
                    else:
                        nc.scalar.copy(dst, ps[:])
                nc.sync.dma_start(
                    out=o1t_d[u][:, ich * 1024:(ich + 1) * 1024],
                    in_=o1sb[u][:, ich * 1024:(ich + 1) * 1024])

            def emit_out2(c, tag):
                ps = pp.tile([128, 256], F32, tag=tag, bufs=1, name="o2_ps")
                jts = [jt for jt in range(2 * c - 1, 2 * c + 3)
                       if 0 <= jt < NI]
                for idx, jt in enumerate(jts):
                    nc.tensor.matmul(
                        ps, lhsT=vtt[:, jt, :],
                        rhs=g_c(jt - 2 * c + 1),
                        start=(idx == 0), stop=(idx == len(jts) - 1))
                nc.vector.tensor_copy(o2sb[:, c * 256:(c + 1) * 256], ps[:])
                if c in (3, NP - 1):
                    cs = slice((c - 3) * 256, (c + 1) * 256)
                    nc.sync.dma_start(out=o2_d[:, cs], in_=o2sb[:, cs])

            # ---- prologue: fp8-DR qk chunks land directly in dots layout;
            # early fills start as soon as k/q chunk 0 are evacuated ----
            emit_qk_chunk(0, k8, 0)
            emit_qk_chunk(128, q8, 0)
            for c4 in range(4):
                dma_xt(c4)
            nc.sync.dma_start(out=cst_sb[:], in_=cst_d[:])
            nc.sync.dma_start(out=rstc_sb[:], in_=rstc_d[:])
            for p in range(2):
                for u in range(2):
                    emit_fill(0, p, u, 0)
            emit_qk_chunk(0, k8, 1)
            emit_qk_chunk(128, q8, 1)
            for p in range(2):
                for u in range(2):
                    emit_fill(0, p, u, 1)
            emit_qk_chunk(0, k8, 2)
            emit_qk_chunk(128, q8, 2)
            emit_qk_chunk(0, k8, 3)
            emit_qk_chunk(128, q8, 3)

            # ---- main: remaining ich0 fills + vt + out2 + F2 ----
            # Tag timelines (serialized by WAR deps within each tag):
            #   A0/A1: qk -> vt -> out2 -> F2 -> out1 (all u/ich, serial)
            #   E/Fq/Bq: warmup + all dots fills (triple-buffered)
            for p in range(NP):
                fp = p + 2
                if fp < NP:
                    for h in range(2):
                        for u in range(2):
                            emit_fill(0, fp, u, h)
                else:
                    # start ich1 fills early to keep the exp engines fed
                    for h in range(2):
                        for u in range(2):
                            emit_fill(1, fp - NP, u, h)
                emit_vt(p, "A0" if p % 2 == 0 else "A1")
            # bf16 stationary copy for the DVE-route out1 (Pool engine;
            # sources the fp8 values so both routes see the same v)
            nc.gpsimd.tensor_copy(vd16[:, :, :, 0:65], vd8[:, :, :, 0:65])
            # ---- out2 + remaining ich1 fills ----
            for s in range(NP):
                emit_out2(s, "A0" if s % 2 == 0 else "A1")
                fp = s + 2
                if fp < NP:
                    for h in range(2):
                        for u in range(2):
                            emit_fill(1, fp, u, h)
            # ---- out1, serial on the A banks: (ich0,u0) .. (ich1,u1) ----
            for ich in range(2):
                for u in range(2):
                    for p in range(NP):
                        emit_out1(ich, u, p)
                    emit_o1_dma(ich, u)

    nc.finalize()
    return nc


_PROGRAM = None


def _get_program():
    global _PROGRAM
    if _PROGRAM is None:
        _PROGRAM = build_program()
    return _PROGRAM


def _host_tables():
    d = np.arange(N, dtype=np.float64)
    g = np.exp(-d / np.e)
    cum = np.cumsum(g)
    j = np.arange(N)
    s = cum[j] + cum[N - 1 - j] - g[0]        # s[j] = sum_k exp(-|j-k|/e)
    smax = s.max()
    # edge corrections s_max/s_j for j-tiles 0 and 15 (elsewhere s_j = s_max
    # to ~1e-10 relative, folded into w2s as the global 1/s_max)
    rstc = np.ascontiguousarray(np.stack(
        [smax / s[0:128], smax / s[N - 128:N]], axis=1).astype(np.float32))
    gi = np.arange(4)[:, None, None]
    p = np.arange(128)[None, :, None]
    f = np.arange(256)[None, None, :]
    gb = np.ascontiguousarray(
        np.exp(-np.abs(f - p - (gi - 1) * 128) / np.e)
        .astype(ml_dtypes.bfloat16))
    return rstc, gb, smax


_TABLES = None


def _tables():
    global _TABLES
    if _TABLES is None:
        _TABLES = _host_tables()
    return _TABLES


def make_in_maps(x, w_qkv, w_out, b_out):
    x = np.asarray(x, np.float32)
    w_qkv = np.asarray(w_qkv, np.float32)
    w_out = np.asarray(w_out, np.float32)
    rstc, gb, smax = _tables()
    w8_scale = 8.0                       # fp8 range scale, undone at evac

    wq_full = w_qkv[0:512]
    wk_full = w_qkv[512:1024]
    wv_full = w_qkv[1024:1536]
    wt_full = w_qkv[1536:2048]

    bf = ml_dtypes.bfloat16
    in_maps = []
    for c in range(NCORES):
        b = c // 4
        h0 = 2 * (c % 4)
        h1 = h0 + 1

        f8 = ml_dtypes.float8_e4m3
        xT = x[b].T                                        # [512, N]
        xt = np.ascontiguousarray(xT.reshape(KT, 128, N).astype(bf))
        xt8 = np.ascontiguousarray(
            xT.reshape(2, 2, 128, N).transpose(2, 0, 1, 3)
            .reshape(128, 4, N).astype(f8))

        # wqk8 cols: [k-lo | k-hi | q-lo | q-hi], each 64 = [u0 | u1]
        def whalf(wfull, h, lohi):
            return wfull[h * 64 + lohi * 32:h * 64 + lohi * 32 + 32].T

        cols = []
        for wfull in (wk_full, wq_full):
            for lohi in range(2):
                cols += [whalf(wfull, h0, lohi), whalf(wfull, h1, lohi)]
        wqk8_ = np.concatenate(cols, axis=1) * w8_scale    # [512, 256]
        wqk8 = np.ascontiguousarray(
            wqk8_.reshape(2, 2, 128, 256).transpose(2, 0, 1, 3)
            .reshape(128, 4, 256).astype(f8))
        wvt_ = np.concatenate(
            [wt_full[h0 * 64:(h0 + 1) * 64].T,
             wt_full[h1 * 64:(h1 + 1) * 64].T,
             wv_full[h0 * 64:(h0 + 1) * 64].T * VSCALE,
             wv_full[h1 * 64:(h1 + 1) * 64].T * VSCALE], axis=1)
        wvt = wvt_.reshape(KT, 128, 256)            # [kt, p, c]
        wvt_cols = wvt.transpose(1, 0, 2).reshape(128, 1024)
        gb_cols = np.asarray(gb, np.float32).transpose(1, 0, 2).reshape(128, 1024)
        cst = np.ascontiguousarray(np.concatenate(
            [wvt_cols, gb_cols], axis=1).astype(bf))
        in_maps.append({
            "xt": xt, "xt8": xt8, "wqk8": wqk8, "cst": cst, "rstc": rstc,
        })
    return in_maps


def combine_outputs(results, w_out, b_out):
    """Host-side unshard: per-core partials -> full [B, N, DIM] output."""
    w_out = np.asarray(w_out, np.float32)
    b_out = np.asarray(b_out, np.float32)
    out = np.zeros((B, N, 512), np.float64)
    for c in range(NCORES):
        r = results[c]
        b = c // 4
        h0 = 2 * (c % 4)
        rstc, gb, smax = _tables()
        o2 = r["o2"].astype(np.float64)                    # [128 ch, N]
        w2 = np.concatenate(
            [w_out[:, h0 * 128 + 64:(h0 + 1) * 128].T,
             w_out[:, (h0 + 1) * 128 + 64:(h0 + 2) * 128].T],
            axis=0).astype(np.float64) / smax
        part = o2.T @ w2                                   # [N, 512]
        for u, h in ((0, h0), (1, h0 + 1)):
            o1 = r[f"o1t{u}"].astype(np.float64)
            o1v = (o1[0:64] / o1[64][None, :]).T            # [N, 64]
            w1 = w_out[:, h * 128:h * 128 + 64].T.astype(np.float64)
            part = part + o1v @ w1
        out[b] += part
    out += b_out[None, None, :].astype(np.float64)
    return out.astype(np.float32)


def kernel(x, w_qkv, w_out, b_out):
    nc = _get_program()
    in_maps = make_in_maps(x, w_qkv, w_out, b_out)
    res = run_bass_kernel_spmd(nc, in_maps, core_ids=list(range(NCORES)))
    return combine_outputs(res.results, w_out, b_out)


def kernel_profiled(x, w_qkv, w_out, b_out):
    out = kernel(x, w_qkv, w_out, b_out)
    return out, None
